# revision 46
# baseline (speedup 1.0000x reference)
"""Trainium2 Bass kernel for 2-layer HGT message passing + sparse gather-dot,
sharded over 8 NeuronCores.

Layout strategy (v2):
 - Nodes of each type are RELABELED host-side by in-degree rank:
   new_id = band*128 + slot, band = rank//128 (80 bands, degree-sorted),
   core(band) = band % 8.  All indices (edges, final queries) are remapped
   through the permutation, so the device never sees it.
 - Edge phase uses a dst-per-partition layout: for a 128-dst block, slot
   (p, j) holds the j-th in-edge of dst p.  J_b = max in-block degree is a
   compile-time constant per block (degree sorting makes bands homogeneous,
   so padding is small).  Padding slots gather a zeroed kv row; the softmax
   denominator is corrected by a host-computed pad count.
 - Per-edge q is a per-partition broadcast (dst == partition); segment
   softmax sum and scatter-add become identity-matmul tile accumulations in
   PSUM on the PE (bf16, 1 cycle/row).
 - v/k/q table columns are permuted to d-major (col d*8+h) host-side so the
   exp-weight broadcast multiply keeps DVE's 2x 16-bit mode; the Wa rows are
   permuted to match.
 - All tables and gathered data are bf16 (512B gather rows = DMA descriptor
   sweet spot).  PSUM accumulation stays f32.
 - Per-core data (edge idx, pad counts, my-node ids, final query idx) are
   inputs; one program serves all 8 cores.  Updated features are AllGathered
   (bf16) per type per layer; each type's collective is issued right after
   its edge direction so the other direction's compute hides it.
"""
import numpy as np

N = 10000
NP = 10240          # padded node count (80 tiles of 128)
NT = NP // 128      # 80 tiles
NCORE = 8
NBLK = NT // NCORE  # 10 blocks (dst tiles) per core
F = 128; HID = 128; H = 8; D = 16; L = 2
EF = 500000
EFC = EF // NCORE   # 62500 final edges per core
FCH = 8             # final tiles per chunk
NFCH = 64           # chunks: 64*8*128 = 65536 >= 62500
FT = NFCH * FCH     # 512 final tiles
ZROW = NP           # zero row in kv table used by padding slots


def _wrap_idx(idx):
    """int index list (len%16==0) -> [128, len//16] int16 in gather format."""
    a = np.asarray(idx, np.int16).reshape(-1, 16).T
    return np.ascontiguousarray(np.tile(a, (8, 1)))


def _blockdiag(a):
    out = np.zeros((HID, HID), np.float32)
    for h in range(H):
        out[h * D:(h + 1) * D, h * D:(h + 1) * D] = a[h]
    return out


# column permutation (h,d) -> d-major (d*8+h)
_PDH = np.zeros(HID, np.int64)
for _h in range(H):
    for _d in range(D):
        _PDH[_d * H + _h] = _h * D + _d   # new col i=d*8+h takes old col h*16+d


def _perm_from_degree(deg):
    """deg[NP] -> perm (old->new), degree-ascending bands dealt round-robin."""
    order = np.argsort(deg, kind="stable")       # order[r] = old id
    perm = np.empty(NP, np.int64)
    r = np.arange(NP)
    perm[order] = r                               # new_id = rank
    return perm


def _prep_edges(ei, perm_s, perm_d):
    """-> per-core dict(idx [128, SJ*8] i16, padc [NBLK,128] f32), J list."""
    s = perm_s[np.asarray(ei[0])]
    d = perm_d[np.asarray(ei[1])]
    band = d // 128
    core = band % NCORE
    blk = band // NCORE
    p = d % 128
    # j-th edge of each dst: stable sort by d, position within group
    order = np.argsort(d, kind="stable")
    ds = d[order]
    cnt = np.bincount(d, minlength=NP)
    starts = np.zeros(NP + 1, np.int64)
    np.cumsum(cnt, out=starts[1:])
    j_of = np.arange(len(ds)) - starts[ds]
    # J per (core, blk): max degree in band
    J = np.zeros((NCORE, NBLK), np.int64)
    for b in range(NT):
        mx = cnt[b * 128:(b + 1) * 128].max()
        J[b % NCORE, b // NCORE] = max(J[b % NCORE, b // NCORE], mx)
    Jb = [max(1, int(J[:, b].max())) for b in range(NBLK)]  # same for all cores
    out = []
    ss = s[order]
    cs = core[order]; bs = blk[order]; ps = p[order]
    for c in range(NCORE):
        idxs = []
        padc = np.zeros((NBLK, 128), np.float32)
        m_c = cs == c
        for b in range(NBLK):
            Jcb = Jb[b]
            A = np.full((Jcb, 128), ZROW, np.int64)
            m = m_c & (bs == b)
            A[j_of[m], ps[m]] = ss[m]
            band_cnt = cnt[(b * NCORE + c) * 128:(b * NCORE + c + 1) * 128]
            # 1e-3 denominator bias keeps zero-degree rows finite (0*1000=0);
            # relative effect on real weights ~1e-3/32, far under tolerance
            padc[b, :] = (Jcb - band_cnt).astype(np.float32) - 1e-3
            idxs.append(_wrap_idx(A.reshape(-1)))
        out.append({"idx": np.ascontiguousarray(np.hstack(idxs)),
                    "padc": padc})
    return out, Jb


def _host_prep(inp):
    f32 = lambda x: np.asarray(x, np.float32)
    ei12 = np.asarray(inp["ei_12"]); ei21 = np.asarray(inp["ei_21"])
    deg1 = np.bincount(np.asarray(ei21[1]), minlength=NP)[:NP]
    deg2 = np.bincount(np.asarray(ei12[1]), minlength=NP)[:NP]
    perm = {1: _perm_from_degree(deg1), 2: _perm_from_degree(deg2)}
    inv = {t: np.argsort(perm[t]) for t in (1, 2)}

    P = {}
    for t, xn, wn, bn in ((1, "x_n1", "W_in1", "b_in1"), (2, "x_n2", "W_in2", "b_in2")):
        x = np.zeros((NP, F), np.float32)
        x[:N] = f32(inp[xn])
        P[f"xT{t}"] = np.ascontiguousarray(x[inv[t]].T)
        P[f"Win{t}"] = f32(inp[wn])
        P[f"binc{t}"] = np.ascontiguousarray(f32(inp[bn]).reshape(HID, 1))
        P[f"binr{t}"] = f32(inp[bn]).reshape(1, HID)
    for t in (1, 2):
        rel = "12" if t == 1 else "21"
        sfx = f"n{t}"
        for l in range(L):
            bd_a = _blockdiag(f32(inp[f"a_rel_{rel}"][l]))
            bd_m = _blockdiag(f32(inp[f"m_rel_{rel}"][l]))
            scale = np.repeat(f32(inp[f"p_rel_{rel}"][l]), D) / np.sqrt(D)
            wk = (f32(inp[f"Wk_{sfx}"][l]) @ bd_a * scale[None, :])[:, _PDH]
            bk = (f32(inp[f"bk_{sfx}"][l]) @ bd_a * scale)[_PDH]
            wv = (f32(inp[f"Wv_{sfx}"][l]) @ bd_m)[:, _PDH]
            bv = (f32(inp[f"bv_{sfx}"][l]) @ bd_m)[_PDH]
            wq = f32(inp[f"Wq_{sfx}"][l])[:, _PDH]
            bq = f32(inp[f"bq_{sfx}"][l])[_PDH]
            P[f"Wtab{t}_l{l}"] = np.ascontiguousarray(
                np.concatenate([wk, wv, wq], axis=1))            # [128, 384]
            P[f"btab{t}_l{l}"] = np.concatenate([bk, bv, bq]).reshape(1, 3 * HID)
            b = 1.0 / (1.0 + np.exp(-float(inp[f"skip_{sfx}"][l])))
            P[f"Wup{t}_l{l}"] = np.ascontiguousarray(b * f32(inp[f"Wa_{sfx}"][l])[_PDH, :])
            P[f"bup{t}_l{l}"] = (b * f32(inp[f"ba_{sfx}"][l])).reshape(1, HID)
            P[f"Ibl{t}_l{l}"] = ((1.0 - b) * np.eye(HID)).astype(np.float32)
    P["ident"] = np.eye(128, dtype=np.float32)
    P["ones1"] = np.ones((1, 128), np.float32)
    packs = [P.pop("Win1"), P.pop("Win2"), P.pop("ident")]
    for t in (1, 2):
        for l in range(L):
            packs.append(P.pop(f"Wtab{t}_l{l}"))
    for t in (1, 2):
        for l in range(L):
            packs.append(P.pop(f"Wup{t}_l{l}"))
            packs.append(P.pop(f"Ibl{t}_l{l}"))
    P["wpack"] = np.ascontiguousarray(np.concatenate(packs, axis=1))
    P["bincp"] = np.ascontiguousarray(
        np.concatenate([P.pop("binc1"), P.pop("binc2")], axis=1))

    e12, J12 = _prep_edges(ei12, perm[1], perm[2])
    e21, J21 = _prep_edges(ei21, perm[2], perm[1])

    # per-core my-node ids (block-major) + final query idx
    eidx = np.asarray(inp["edge_index"])
    mi_all = perm[1][eidx[0]]; di_all = perm[2][eidx[1]]
    percore = []
    for c in range(NCORE):
        my = ((np.arange(NBLK) * NCORE + c)[:, None] * 128
              + np.arange(128)[None, :]).reshape(-1)    # (b*8+c)*128 + p
        mi = np.zeros(FT * 128, np.int64); di = np.zeros(FT * 128, np.int64)
        mi[:EFC] = mi_all[c * EFC:(c + 1) * EFC]
        di[:EFC] = di_all[c * EFC:(c + 1) * EFC]
        percore.append({"myid": _wrap_idx(my),
                        "fmi": _wrap_idx(mi), "fdi": _wrap_idx(di)})
    return P, e12, e21, percore, tuple(J12), tuple(J21)


def _build(J12, J21, phases=4, bias_zero=False):
    import concourse.bass as bass
    import concourse.mybir as mybir
    from concourse import bacc, tile, library_config
    from concourse.bass import broadcast_tensor_aps

    dt = mybir.dt
    AF = mybir.ActivationFunctionType
    ALU = mybir.AluOpType
    BF = dt.bfloat16
    nc = bacc.Bacc("TRN2")

    SJ8 = {d: sum(J) * 8 for d, J in (("12", J12), ("21", J21))}
    JMAX = {"12": max(J12), "21": max(J21)}
    JL = {"12": J12, "21": J21}

    def inP(name, shape, dty=dt.float32):
        return nc.declare_dram_parameter(name, list(shape), dty, isOutput=False)

    WCOLS = 128 * 3 + 384 * L * 2 + 256 * L * 2
    pr = {}
    for t in (1, 2):
        pr[f"xT{t}"] = inP(f"xT{t}", [128, NP])
        pr[f"binr{t}"] = inP(f"binr{t}", [1, 128])
        for l in range(L):
            for nm, sh in (("btab", [1, 384]), ("bup", [1, 128])):
                pr[f"{nm}{t}_l{l}"] = inP(f"{nm}{t}_l{l}", sh)
    pr["wpack"] = inP("wpack", [128, WCOLS])
    pr["bincp"] = inP("bincp", [128, 2])
    pr["ones1"] = inP("ones1", [1, 128])
    for dname in ("12", "21"):
        pr[f"ei{dname}"] = inP(f"ei{dname}", [128, SJ8[dname]], dt.int16)
        pr[f"pc{dname}"] = inP(f"pc{dname}", [NBLK, 128])
    pr["myid"] = inP("myid", [128, NBLK * 8], dt.int16)
    pr["fmi"] = inP("fmi", [128, FT * 8], dt.int16)
    pr["fdi"] = inP("fdi", [128, FT * 8], dt.int16)
    y_out = nc.declare_dram_parameter("y", [128, FT], dt.float32, isOutput=True)

    kv = {t: nc.dram_tensor(f"kv{t}", [NP + 128, 256], BF) for t in (1, 2)}
    qt = {t: nc.dram_tensor(f"qt{t}", [NP, 128], BF) for t in (1, 2)}
    xt0 = {t: nc.dram_tensor(f"xt0{t}", [NP, 128], BF) for t in (1, 2)}
    emed = {t: nc.dram_tensor(f"emed{t}", [NP, 256], BF) for t in (1, 2)}
    agin = {t: nc.dram_tensor(f"agin{t}", [NBLK * 128, 128], BF) for t in (1, 2)}
    agout = {t: nc.dram_tensor(f"agout{t}", [NCORE, NBLK * 128, 128], BF,
                               addr_space="Shared") for t in (1, 2)}

    from contextlib import ExitStack
    with tile.TileContext(nc) as tc, ExitStack() as stack:
        nc.gpsimd.load_library(library_config.mlp)
        cp = stack.enter_context(tc.tile_pool(name="const", bufs=1))
        W = {}
        # f32 weights -> bf16 SBUF copies
        with tc.tile_pool(name="wld", bufs=1) as wp:
            WCOLS = pr["wpack"].shape[1]
            wf = wp.tile([128, WCOLS], dt.float32, tag="wf")
            nc.sync.dma_start(wf[:], pr["wpack"][:])
            wb = cp.tile([128, WCOLS], BF, tag="wb")
            nc.vector.tensor_copy(wb[:], wf[:])
            off = 0
            names = ["Win1", "Win2", "ident"]
            widths = [128, 128, 128]
            for t in (1, 2):
                for l in range(L):
                    names.append(f"Wtab{t}_l{l}"); widths.append(384)
            for t in (1, 2):
                for l in range(L):
                    names.append(f"Wup{t}_l{l}"); widths.append(128)
                    names.append(f"Ibl{t}_l{l}"); widths.append(128)
            for nm, wd in zip(names, widths):
                W[nm] = wb[:, off:off + wd]
                off += wd
            bc = cp.tile([128, 2], dt.float32, tag="bincp")
            nc.sync.dma_start(bc[:], pr["bincp"][:])
            W["binc1"] = bc[:, 0:1]
            W["binc2"] = bc[:, 1:2]
            if not bias_zero:
                for k in ("ones1", "binr1", "binr2",
                          *(f"btab{t}_l{l}" for t in (1, 2) for l in range(L)),
                          *(f"bup{t}_l{l}" for t in (1, 2) for l in range(L))):
                    p = pr[k]
                    tf2 = wp.tile(list(p.shape), dt.float32,
                                  tag="wf1" + str(list(p.shape)), bufs=2)
                    nc.sync.dma_start(tf2[:], p[:])
                    t_ = cp.tile(list(p.shape), BF, tag=k)
                    nc.vector.tensor_copy(t_[:], tf2[:])
                    W[k] = t_
        for dname in ("12", "21"):
            t_ = cp.tile([128, SJ8[dname]], dt.int16, tag=f"ei{dname}")
            nc.sync.dma_start(t_[:], pr[f"ei{dname}"][:])
            W[f"ei{dname}"] = t_
            t_ = cp.tile([128, NBLK], dt.float32, tag=f"pc{dname}")
            nc.sync.dma_start(t_[:], pr[f"pc{dname}"].rearrange("b p -> p b"))
            W[f"pc{dname}"] = t_
        t_ = cp.tile([128, NBLK * 8], dt.int16, tag="myid")
        nc.sync.dma_start(t_[:], pr["myid"][:])
        W["myid"] = t_

        def tt(eng, out, a, b, op):
            a2, b2 = broadcast_tensor_aps(a, b)
            eng.tensor_tensor(out, a2, b2, op)

        def cpy(eng, dst, src):
            if eng is nc.scalar:
                eng.copy(dst, src)
            else:
                eng.tensor_copy(dst, src)

        def gat8(out_t, table, idx_sb, base8, ntiles, elem):
            # dma_gather hardware cap: 1024 indices (8 tiles) per call
            for g0 in range(0, ntiles, 8):
                gn = min(8, ntiles - g0)
                nc.gpsimd.dma_gather(
                    out_t[:, g0:g0 + gn, :], table[:, :],
                    idx_sb[:, base8 + g0 * 8:base8 + (g0 + gn) * 8],
                    gn * 128, gn * 128, elem)

        # persistent feature tiles
        xc = {t: cp.tile([128, NP], BF, tag=f"xc{t}", name=f"xc{t}") for t in (1, 2)}
        xrow = {t: cp.tile([128, NBLK, 128], BF, tag=f"xrow{t}", name=f"xrow{t}") for t in (1, 2)}
        xnew = {t: cp.tile([128, NBLK, 128], BF, tag=f"xnew{t}", name=f"xnew{t}") for t in (1, 2)}
        qmy = {t: cp.tile([128, NBLK, 128], BF, tag=f"qmy{t}", name=f"qmy{t}") for t in (1, 2)}

        # ---------- phase 0: input projection ----------
        with tc.tile_pool(name="p0", bufs=1) as p0, \
             tc.tile_pool(name="p0b", bufs=2) as p0b, \
             tc.tile_pool(name="p0s", bufs=2) as p0s, \
             tc.tile_pool(name="p0ps", bufs=4, space="PSUM") as p0p:
            ts_ = (1, 2) if phases >= 0 else ()
            xb = {}
            for t in ts_:
                xb[t] = p0b.tile([128, NP], BF, tag="xb", name=f"xb{t}")
                for hf in range(4):
                    xf = p0.tile([128, NP // 4], dt.float32, tag="xf", bufs=2)
                    nc.sync.dma_start(xf[:], pr[f"xT{t}"][:, bass.ts(hf, NP // 4)])
                    if t == 1:
                        nc.vector.tensor_copy(xb[t][:, bass.ts(hf, NP // 4)], xf[:])
                    else:
                        nc.scalar.copy(xb[t][:, bass.ts(hf, NP // 4)], xf[:])
            for t in ts_:
                # transposed projection -> xc (x1T)
                for j in range(NP // 512):
                    ps = p0p.tile([128, 512], dt.float32, tag="psP")
                    nc.tensor.matmul(ps[:], W[f"Win{t}"], xb[t][:, bass.ts(j, 512)],
                                     start=True, stop=True)
                    nc.scalar.activation(xc[t][:, bass.ts(j, 512)], ps[:],
                                         AF.Relu, bias=W[f"binc{t}"])
            for t in ts_:
                # row projection -> xt0 table (for my-row gather)
                rstage = p0s.tile([128, NT // 2, 128], BF, tag="rst")
                for hh in range(2):
                    for i4 in range(NT // 8):
                        ps = p0p.tile([128, 4, 128], dt.float32, tag="psR")
                        for k in range(4):
                            g = hh * (NT // 2) + i4 * 4 + k
                            nc.tensor.matmul(ps[:, k, :], xb[t][:, bass.ts(g, 128)],
                                             W[f"Win{t}"], start=True,
                                             stop=bias_zero)
                            if not bias_zero:
                                nc.tensor.matmul(ps[:, k, :], W["ones1"][:1, :],
                                                 W[f"binr{t}"][:1, :],
                                                 start=False, stop=True)
                        nc.scalar.activation(
                            rstage[:, i4 * 4:(i4 + 1) * 4, :], ps[:], AF.Relu)
                    nc.sync.dma_start(
                        xt0[t].rearrange("(a p) c -> p a c", p=128)[:, hh * 40:(hh + 1) * 40, :],
                        rstage[:])
                # zero pad rows of kv
                z = p0s.tile([128, 256], BF, tag="z")
                nc.vector.memset(z[:], 0.0)
                nc.sync.dma_start(kv[t][NP:NP + 128, :], z[:])
            # gather my x0 rows
            for t in ts_:
                gat8(xrow[t], xt0[t], W["myid"], 0, NBLK, 128)

        def emit_tables(t, l):
            with tc.tile_pool(name="tab", bufs=2) as tp, \
                 tc.tile_pool(name="tabps", bufs=4, space="PSUM") as tps:
                NH = NT // 2
                for hh in range(2):
                    kvq = tp.tile([128, NH, 384], BF, tag="kvq")
                    for i in range(NH):
                        g = hh * NH + i
                        ps = tps.tile([128, 384], dt.float32, tag="psT")
                        nc.tensor.matmul(ps[:], xc[t][:, bass.ts(g, 128)],
                                         W[f"Wtab{t}_l{l}"], start=True,
                                         stop=bias_zero)
                        if not bias_zero:
                            nc.tensor.matmul(ps[:], W["ones1"][:1, :],
                                             W[f"btab{t}_l{l}"][:1, :],
                                             start=False, stop=True)
                        cpy(nc.scalar if i % 2 == 0 else nc.vector,
                            kvq[:, i, :], ps[:])
                    nc.sync.dma_start(
                        kv[t].rearrange("(a p) c -> p a c", p=128)[:, hh * NH:(hh + 1) * NH, :],
                        kvq[:, :, 0:256])
                    nc.sync.dma_start(
                        qt[t].rearrange("(a p) c -> p a c", p=128)[:, hh * NH:(hh + 1) * NH, :],
                        kvq[:, :, 256:384])
                gat8(qmy[t], qt[t], W["myid"], 0, NBLK, 128)

        for l in range(L if phases >= 1 else 0):
            # ---------- tables ----------
            # (type-2 tables of layer l>0 were emitted during layer l-1's
            # post phase, hiding the second collective)
            if l == 0:
                emit_tables(2, 0)
            emit_tables(1, l)

            # ---------- edge phase ----------
            CH = 32
            for dname, st, dtt in ((("12", 1, 2), ("21", 2, 1)) if phases >= 2 else ()):
                chunks = []
                off8 = 0
                for b in range(NBLK):
                    J = JL[dname][b]
                    nch = (J + CH - 1) // CH
                    for ci in range(nch):
                        j0 = ci * CH
                        jn = min(CH, J - j0)
                        chunks.append((b, off8 + j0 * 8, jn,
                                       ci == 0, ci == nch - 1))
                    off8 += J * 8
                with tc.tile_pool(name="eg", bufs=4) as eg, \
                     tc.tile_pool(name="epw", bufs=3) as epw, \
                     tc.tile_pool(name="ew", bufs=2) as ew, \
                     tc.tile_pool(name="eps", bufs=2, space="PSUM") as eps, \
                     tc.tile_pool(name="epsg", bufs=2, space="PSUM") as epsg, \
                     tc.tile_pool(name="epsu", bufs=2, space="PSUM") as epsu:
                    psG = sacc = None
                    for (b, coff8, jn, first, last) in chunks:
                        kvg = eg.tile([128, CH, 256], BF, tag="kvg")
                        gat8(kvg, kv[st], W[f"ei{dname}"], coff8, jn, 256)
                        prod = epw.tile([128, CH, 128], BF, tag="pw", name="prod")
                        tt(nc.vector, prod[:, 0:jn, :], kvg[:, 0:jn, 0:128],
                           qmy[dtt][:, b:b + 1, :], ALU.mult)
                        # alpha[p, j, h] = sum_d prod[p, j, d*8+h]: DVE tree
                        with nc.allow_low_precision(reason="alpha bf16 tree"):
                            w_ = D
                            while w_ > 1:
                                h_ = w_ // 2
                                tt(nc.vector, prod[:, 0:jn, 0:h_ * 8],
                                   prod[:, 0:jn, 0:h_ * 8],
                                   prod[:, 0:jn, h_ * 8:w_ * 8], ALU.add)
                                w_ = h_
                        eB = ew.tile([128, CH, 8], BF, tag="eB")
                        nc.scalar.activation(eB[:, 0:jn, :], prod[:, 0:jn, 0:8],
                                             AF.Exp)
                        # wv[p, j, d*8+h] = v * e  (2x: d-major v, h innermost)
                        wv = epw.tile([128, CH, 128], BF, tag="pw", name="wv")
                        tt(nc.vector,
                           wv[:, 0:jn, :].rearrange("p j (d h) -> p j d h", h=8),
                           kvg[:, 0:jn, 128:256].rearrange("p j (d h) -> p j d h", h=8),
                           eB[:, 0:jn, :].rearrange("p j (d2 h) -> p j d2 h", d2=1),
                           ALU.mult)
                        # s tree over j (in-place on eB), bf16 accum
                        with nc.allow_low_precision(reason="softmax denom bf16 tree"):
                            Jc = jn
                            while Jc > 1:
                                h1 = (Jc + 1) // 2
                                tt(nc.vector, eB[:, 0:Jc - h1, :], eB[:, 0:Jc - h1, :],
                                   eB[:, h1:Jc, :], ALU.add)
                                Jc = h1
                        if first:
                            sacc = ew.tile([128, 8], dt.float32, tag="sacc")
                            nc.vector.tensor_copy(sacc[:], eB[:, 0, :])
                        else:
                            tt(nc.vector, sacc[:], sacc[:], eB[:, 0, :], ALU.add)
                        # agg[p, dh] += sum_j wv: 4-tile-packed identity matmuls
                        if first:
                            psG = epsg.tile([128, 4, 128], dt.float32, tag="psG")
                        nst = (jn + 3) // 4
                        for g in range(nst):
                            gw = min(4, jn - g * 4)
                            nc.tensor.matmul(psG[:, 0:gw, :], W["ident"],
                                             wv[:, g * 4:g * 4 + gw, :],
                                             start=(first and g == 0),
                                             stop=(last and g == nst - 1))
                        if not last:
                            continue
                        sden = ew.tile([128, 8], dt.float32, tag="sden")
                        tt(nc.vector, sden[:, :], sacc[:],
                           W[f"pc{dname}"][:, b:b + 1], ALU.subtract)
                        rs = ew.tile([128, 8], dt.float32, tag="rs")
                        nc.vector.reciprocal(rs[:], sden[:])
                        agf = ew.tile([128, 128], dt.float32, tag="agf")
                        nc.vector.tensor_reduce(
                            agf[:], psG[:].rearrange("p r c -> p c r"),
                            mybir.AxisListType.X, ALU.add)
                        # normalize + gelu
                        gn = ew.tile([128, 128], BF, tag="gn")
                        tt(nc.vector, gn[:].rearrange("p (d h) -> p d h", h=8),
                           agf[:].rearrange("p (d h) -> p d h", h=8),
                           rs[:].rearrange("p (d2 h) -> p d2 h", d2=1), ALU.mult)
                        gb = ew.tile([128, 128], BF, tag="gb")
                        nc.scalar.activation(gb[:], gn[:], AF.Gelu)
                        # update: x_new = gelu^T @ Wup + bup + Ibl @ x_old
                        trp = eps.tile([128, 128], BF, tag="trp")
                        nc.tensor.transpose(trp[:], gb[:], W["ident"])
                        gT = ew.tile([128, 128], BF, tag="gT")
                        nc.scalar.copy(gT[:], trp[:])
                        psU = epsu.tile([128, 128], dt.float32, tag="psU")
                        nc.tensor.matmul(psU[:], gT[:], W[f"Wup{dtt}_l{l}"],
                                         start=True, stop=False)
                        if not bias_zero:
                            nc.tensor.matmul(psU[:], W["ones1"][:1, :],
                                             W[f"bup{dtt}_l{l}"][:1, :],
                                             start=False, stop=False)
                        nc.tensor.matmul(psU[:], W[f"Ibl{dtt}_l{l}"],
                                         xrow[dtt][:, b, :], start=False, stop=True)
                        nc.vector.tensor_copy(xnew[dtt][:, b, :], psU[:])
                    nc.sync.dma_start(
                        agin[dtt].rearrange("(b p) c -> p b c", p=128),
                        xnew[dtt][:])
                # per-type allgather right after its edge dir: the second
                # dir's edge compute hides the first collective
                if phases >= 3:
                    nc.gpsimd.collective_compute(
                        "AllGather", mybir.AluOpType.bypass,
                        ins=[agin[dtt][:]], outs=[agout[dtt][:]],
                        replica_groups=[list(range(NCORE))])
            # post per type after both dirs (type 2 first: its collective
            # finished during dir 21's edge compute)
            post_types = (2, 1) if phases >= 3 else ()
            for t in post_types:
                with tc.tile_pool(name="post", bufs=1) as pp, \
                     tc.tile_pool(name="postps", bufs=4, space="PSUM") as ppp:
                    stage = pp.tile([128, NT, 128], BF, tag="stage")
                    sv = stage[:].rearrange("p (b r) c -> p b r c", r=NCORE)
                    for r in range(NCORE):
                        nc.sync.dma_start(
                            sv[:, :, r, :],
                            agout[t][r].rearrange("(b p) c -> p b c", p=128))
                    nc.sync.dma_start(
                        emed[t].rearrange("(g p) c -> p g c", p=128)[:, :, l * 128:(l + 1) * 128],
                        stage[:])
                    if l + 1 < L:
                        for q4 in range(NT // 4):
                            ptr = ppp.tile([128, 512], BF, tag="ptr")
                            for k in range(4):
                                nc.tensor.transpose(ptr[:, bass.ts(k, 128)],
                                                    stage[:, q4 * 4 + k, :], W["ident"])
                            cpy(nc.scalar if q4 % 2 == 0 else nc.vector,
                                xc[t][:, bass.ts(q4, 512)], ptr[:])
                if t == 2 and l + 1 < L:
                    # type-2 tables of the next layer: overlaps the type-1
                    # collective still in flight
                    emit_tables(2, l + 1)
                if t == 2 and l + 1 == L:
                    # final-phase index loads ahead of post-1's
                    # collective-gated DMAs, so ed-gathers can prefetch
                    # during the last collective
                    fip = stack.enter_context(tc.tile_pool(name="fidx", bufs=1))
                    for k in ("fmi", "fdi"):
                        t_ = fip.tile([128, FT * 8], dt.int16, tag=k, name=k)
                        nc.sync.dma_start(t_[:], pr[k][:])
                        W[k] = t_
            # my rows for next layer's skip = locally computed x_new
            xrow, xnew = xnew, xrow

        # ---------- final gather-dot ----------
        with tc.tile_pool(name="fin", bufs=6) as fp, \
             tc.tile_pool(name="finps", bufs=4, space="PSUM") as fps, \
             tc.tile_pool(name="ybuf", bufs=1) as yp:
            ysb = yp.tile([128, FT], dt.float32, tag="ysb")
            if phases < 4:
                nc.vector.memset(ysb[:], 0.0)
            for ch in range(NFCH if phases >= 4 else 0):
                em = fp.tile([128, FCH, 256], BF, tag="em")
                ed = fp.tile([128, FCH, 256], BF, tag="ed")
                gat8(ed, emed[2], W["fdi"], ch * FCH * 8, FCH, 256)
                gat8(em, emed[1], W["fmi"], ch * FCH * 8, FCH, 256)
                pb = fp.tile([128, FCH, 256], BF, tag="pb")
                tt(nc.vector, pb[:], em[:], ed[:], ALU.mult)
                # slab-sum on PE: psY[p, t, i] = sum_g pb[p, t, g*16+i]
                psY = fps.tile([128, FCH, 16], dt.float32, tag="psY")
                for g in range(16):
                    nc.tensor.matmul(psY[:], W["ident"],
                                     pb[:, :, bass.ts(g, 16)],
                                     start=(g == 0), stop=(g == 15))
                nc.vector.tensor_reduce(
                    ysb[:, ch * FCH:(ch + 1) * FCH], psY[:],
                    mybir.AxisListType.X, ALU.add)
            nc.sync.dma_start(y_out[:, :], ysb[:])
    nc.compile()
    return nc


_CACHE = {}


def kernel(**inputs):
    from concourse.bass_utils import run_bass_kernel_spmd
    P, e12, e21, percore, J12, J21 = _host_prep(inputs)
    bz = all(not np.any(np.asarray(inputs[k]))
             for k in inputs if k.startswith("b"))
    key = (J12, J21, bz)
    if key not in _CACHE:
        _CACHE[key] = _build(J12, J21, bias_zero=bz)
    nc = _CACHE[key]
    in_maps = []
    for c in range(NCORE):
        m = dict(P)
        m["ei12"] = e12[c]["idx"]; m["pc12"] = e12[c]["padc"]
        m["ei21"] = e21[c]["idx"]; m["pc21"] = e21[c]["padc"]
        m["myid"] = percore[c]["myid"]
        m["fmi"] = percore[c]["fmi"]; m["fdi"] = percore[c]["fdi"]
        in_maps.append(m)
    res = run_bass_kernel_spmd(nc, in_maps, list(range(NCORE)))
    ys = []
    for c in range(NCORE):
        yc = res.results[c]["y"]          # [128, FT]
        ys.append(yc.T.ravel()[:EFC])
    return np.concatenate(ys).astype(np.float32).reshape(EF, 1)


# revision 47
# speedup vs baseline: 1.0274x; 1.0274x over previous
"""Trainium2 Bass kernel for 2-layer HGT message passing + sparse gather-dot,
sharded over 8 NeuronCores.

Layout strategy (v2):
 - Nodes of each type are RELABELED host-side by in-degree rank:
   new_id = band*128 + slot, band = rank//128 (80 bands, degree-sorted),
   core(band) = band % 8.  All indices (edges, final queries) are remapped
   through the permutation, so the device never sees it.
 - Edge phase uses a dst-per-partition layout: for a 128-dst block, slot
   (p, j) holds the j-th in-edge of dst p.  J_b = max in-block degree is a
   compile-time constant per block (degree sorting makes bands homogeneous,
   so padding is small).  Padding slots gather a zeroed kv row; the softmax
   denominator is corrected by a host-computed pad count.
 - Per-edge q is a per-partition broadcast (dst == partition); segment
   softmax sum and scatter-add become identity-matmul tile accumulations in
   PSUM on the PE (bf16, 1 cycle/row).
 - v/k/q table columns are permuted to d-major (col d*8+h) host-side so the
   exp-weight broadcast multiply keeps DVE's 2x 16-bit mode; the Wa rows are
   permuted to match.
 - All tables and gathered data are bf16 (512B gather rows = DMA descriptor
   sweet spot).  PSUM accumulation stays f32.
 - Per-core data (edge idx, pad counts, my-node ids, final query idx) are
   inputs; one program serves all 8 cores.  Updated features are AllGathered
   (bf16) per type per layer; each type's collective is issued right after
   its edge direction so the other direction's compute hides it.
"""
import numpy as np

N = 10000
NP = 10240          # padded node count (80 tiles of 128)
NT = NP // 128      # 80 tiles
NCORE = 8
NBLK = NT // NCORE  # 10 blocks (dst tiles) per core
F = 128; HID = 128; H = 8; D = 16; L = 2
EF = 500000
EFC = EF // NCORE   # 62500 final edges per core
FCH = 8             # final tiles per chunk
NFCH = 64           # chunks: 64*8*128 = 65536 >= 62500
FT = NFCH * FCH     # 512 final tiles
ZROW = NP           # zero row in kv table used by padding slots


def _wrap_idx(idx):
    """int index list (len%16==0) -> [128, len//16] int16 in gather format."""
    a = np.asarray(idx, np.int16).reshape(-1, 16).T
    return np.ascontiguousarray(np.tile(a, (8, 1)))


def _blockdiag(a):
    out = np.zeros((HID, HID), np.float32)
    for h in range(H):
        out[h * D:(h + 1) * D, h * D:(h + 1) * D] = a[h]
    return out


# column permutation (h,d) -> d-major (d*8+h)
_PDH = np.zeros(HID, np.int64)
for _h in range(H):
    for _d in range(D):
        _PDH[_d * H + _h] = _h * D + _d   # new col i=d*8+h takes old col h*16+d


def _perm_from_degree(deg):
    """deg[NP] -> perm (old->new), degree-ascending bands dealt round-robin."""
    order = np.argsort(deg, kind="stable")       # order[r] = old id
    perm = np.empty(NP, np.int64)
    r = np.arange(NP)
    perm[order] = r                               # new_id = rank
    return perm


def _prep_edges(ei, perm_s, perm_d):
    """-> per-core dict(idx [128, SJ*8] i16, padc [NBLK,128] f32), J list."""
    s = perm_s[np.asarray(ei[0])]
    d = perm_d[np.asarray(ei[1])]
    band = d // 128
    core = band % NCORE
    blk = band // NCORE
    p = d % 128
    # j-th edge of each dst: stable sort by d, position within group
    order = np.argsort(d, kind="stable")
    ds = d[order]
    cnt = np.bincount(d, minlength=NP)
    starts = np.zeros(NP + 1, np.int64)
    np.cumsum(cnt, out=starts[1:])
    j_of = np.arange(len(ds)) - starts[ds]
    # J per (core, blk): max degree in band
    J = np.zeros((NCORE, NBLK), np.int64)
    for b in range(NT):
        mx = cnt[b * 128:(b + 1) * 128].max()
        J[b % NCORE, b // NCORE] = max(J[b % NCORE, b // NCORE], mx)
    Jb = [max(1, int(J[:, b].max())) for b in range(NBLK)]  # same for all cores
    out = []
    ss = s[order]
    cs = core[order]; bs = blk[order]; ps = p[order]
    for c in range(NCORE):
        idxs = []
        padc = np.zeros((NBLK, 128), np.float32)
        m_c = cs == c
        for b in range(NBLK):
            Jcb = Jb[b]
            A = np.full((Jcb, 128), ZROW, np.int64)
            m = m_c & (bs == b)
            A[j_of[m], ps[m]] = ss[m]
            band_cnt = cnt[(b * NCORE + c) * 128:(b * NCORE + c + 1) * 128]
            # 1e-3 denominator bias keeps zero-degree rows finite (0*1000=0);
            # relative effect on real weights ~1e-3/32, far under tolerance
            padc[b, :] = (Jcb - band_cnt).astype(np.float32) - 1e-3
            idxs.append(_wrap_idx(A.reshape(-1)))
        out.append({"idx": np.ascontiguousarray(np.hstack(idxs)),
                    "padc": padc})
    return out, Jb


def _host_prep(inp):
    f32 = lambda x: np.asarray(x, np.float32)
    ei12 = np.asarray(inp["ei_12"]); ei21 = np.asarray(inp["ei_21"])
    deg1 = np.bincount(np.asarray(ei21[1]), minlength=NP)[:NP]
    deg2 = np.bincount(np.asarray(ei12[1]), minlength=NP)[:NP]
    perm = {1: _perm_from_degree(deg1), 2: _perm_from_degree(deg2)}
    inv = {t: np.argsort(perm[t]) for t in (1, 2)}

    P = {}
    for t, xn, wn, bn in ((1, "x_n1", "W_in1", "b_in1"), (2, "x_n2", "W_in2", "b_in2")):
        x = np.zeros((NP, F), np.float32)
        x[:N] = f32(inp[xn])
        P[f"xT{t}"] = np.ascontiguousarray(x[inv[t]].T)
        P[f"Win{t}"] = f32(inp[wn])
        P[f"binc{t}"] = np.ascontiguousarray(f32(inp[bn]).reshape(HID, 1))
        P[f"binr{t}"] = f32(inp[bn]).reshape(1, HID)
    for t in (1, 2):
        rel = "12" if t == 1 else "21"
        sfx = f"n{t}"
        for l in range(L):
            bd_a = _blockdiag(f32(inp[f"a_rel_{rel}"][l]))
            bd_m = _blockdiag(f32(inp[f"m_rel_{rel}"][l]))
            scale = np.repeat(f32(inp[f"p_rel_{rel}"][l]), D) / np.sqrt(D)
            wk = (f32(inp[f"Wk_{sfx}"][l]) @ bd_a * scale[None, :])[:, _PDH]
            bk = (f32(inp[f"bk_{sfx}"][l]) @ bd_a * scale)[_PDH]
            wv = (f32(inp[f"Wv_{sfx}"][l]) @ bd_m)[:, _PDH]
            bv = (f32(inp[f"bv_{sfx}"][l]) @ bd_m)[_PDH]
            wq = f32(inp[f"Wq_{sfx}"][l])[:, _PDH]
            bq = f32(inp[f"bq_{sfx}"][l])[_PDH]
            P[f"Wtab{t}_l{l}"] = np.ascontiguousarray(
                np.concatenate([wk, wv, wq], axis=1))            # [128, 384]
            P[f"btab{t}_l{l}"] = np.concatenate([bk, bv, bq]).reshape(1, 3 * HID)
            b = 1.0 / (1.0 + np.exp(-float(inp[f"skip_{sfx}"][l])))
            P[f"Wup{t}_l{l}"] = np.ascontiguousarray(b * f32(inp[f"Wa_{sfx}"][l])[_PDH, :])
            P[f"bup{t}_l{l}"] = (b * f32(inp[f"ba_{sfx}"][l])).reshape(1, HID)
            P[f"Ibl{t}_l{l}"] = ((1.0 - b) * np.eye(HID)).astype(np.float32)
    P["ident"] = np.eye(128, dtype=np.float32)
    P["ones1"] = np.ones((1, 128), np.float32)
    packs = [P.pop("Win1"), P.pop("Win2"), P.pop("ident")]
    for t in (1, 2):
        for l in range(L):
            packs.append(P.pop(f"Wtab{t}_l{l}"))
    for t in (1, 2):
        for l in range(L):
            packs.append(P.pop(f"Wup{t}_l{l}"))
            packs.append(P.pop(f"Ibl{t}_l{l}"))
    P["wpack"] = np.ascontiguousarray(np.concatenate(packs, axis=1))
    P["bincp"] = np.ascontiguousarray(
        np.concatenate([P.pop("binc1"), P.pop("binc2")], axis=1))

    e12, J12 = _prep_edges(ei12, perm[1], perm[2])
    e21, J21 = _prep_edges(ei21, perm[2], perm[1])

    # per-core my-node ids (block-major) + final query idx
    eidx = np.asarray(inp["edge_index"])
    mi_all = perm[1][eidx[0]]; di_all = perm[2][eidx[1]]
    percore = []
    for c in range(NCORE):
        my = ((np.arange(NBLK) * NCORE + c)[:, None] * 128
              + np.arange(128)[None, :]).reshape(-1)    # (b*8+c)*128 + p
        mi = np.zeros(FT * 128, np.int64); di = np.zeros(FT * 128, np.int64)
        mi[:EFC] = mi_all[c * EFC:(c + 1) * EFC]
        di[:EFC] = di_all[c * EFC:(c + 1) * EFC]
        percore.append({"myid": _wrap_idx(my),
                        "fmi": _wrap_idx(mi), "fdi": _wrap_idx(di)})
    return P, e12, e21, percore, tuple(J12), tuple(J21)


def _build(J12, J21, phases=4, bias_zero=False):
    import concourse.bass as bass
    import concourse.mybir as mybir
    from concourse import bacc, tile, library_config
    from concourse.bass import broadcast_tensor_aps

    dt = mybir.dt
    AF = mybir.ActivationFunctionType
    ALU = mybir.AluOpType
    BF = dt.bfloat16
    nc = bacc.Bacc("TRN2")

    SJ8 = {d: sum(J) * 8 for d, J in (("12", J12), ("21", J21))}
    JMAX = {"12": max(J12), "21": max(J21)}
    JL = {"12": J12, "21": J21}

    def inP(name, shape, dty=dt.float32):
        return nc.declare_dram_parameter(name, list(shape), dty, isOutput=False)

    WCOLS = 128 * 3 + 384 * L * 2 + 256 * L * 2
    pr = {}
    for t in (1, 2):
        pr[f"xT{t}"] = inP(f"xT{t}", [128, NP])
        pr[f"binr{t}"] = inP(f"binr{t}", [1, 128])
        for l in range(L):
            for nm, sh in (("btab", [1, 384]), ("bup", [1, 128])):
                pr[f"{nm}{t}_l{l}"] = inP(f"{nm}{t}_l{l}", sh)
    pr["wpack"] = inP("wpack", [128, WCOLS])
    pr["bincp"] = inP("bincp", [128, 2])
    pr["ones1"] = inP("ones1", [1, 128])
    for dname in ("12", "21"):
        pr[f"ei{dname}"] = inP(f"ei{dname}", [128, SJ8[dname]], dt.int16)
        pr[f"pc{dname}"] = inP(f"pc{dname}", [NBLK, 128])
    pr["myid"] = inP("myid", [128, NBLK * 8], dt.int16)
    pr["fmi"] = inP("fmi", [128, FT * 8], dt.int16)
    pr["fdi"] = inP("fdi", [128, FT * 8], dt.int16)
    y_out = nc.declare_dram_parameter("y", [128, FT], dt.float32, isOutput=True)

    kv = {t: nc.dram_tensor(f"kv{t}", [NP + 128, 256], BF) for t in (1, 2)}
    qt = {t: nc.dram_tensor(f"qt{t}", [NP, 128], BF) for t in (1, 2)}
    xt0 = {t: nc.dram_tensor(f"xt0{t}", [NP, 128], BF) for t in (1, 2)}
    emed = {t: nc.dram_tensor(f"emed{t}", [NP, 256], BF) for t in (1, 2)}
    agin = {t: nc.dram_tensor(f"agin{t}", [NBLK * 128, 128], BF) for t in (1, 2)}
    agout = {t: nc.dram_tensor(f"agout{t}", [NCORE, NBLK * 128, 128], BF,
                               addr_space="Shared") for t in (1, 2)}

    from contextlib import ExitStack
    with tile.TileContext(nc) as tc, ExitStack() as stack:
        nc.gpsimd.load_library(library_config.mlp)
        cp = stack.enter_context(tc.tile_pool(name="const", bufs=1))
        W = {}
        # f32 weights -> bf16 SBUF copies
        with tc.tile_pool(name="wld", bufs=1) as wp:
            WCOLS = pr["wpack"].shape[1]
            wf = wp.tile([128, WCOLS], dt.float32, tag="wf")
            nc.sync.dma_start(wf[:], pr["wpack"][:])
            wb = cp.tile([128, WCOLS], BF, tag="wb")
            nc.vector.tensor_copy(wb[:], wf[:])
            off = 0
            names = ["Win1", "Win2", "ident"]
            widths = [128, 128, 128]
            for t in (1, 2):
                for l in range(L):
                    names.append(f"Wtab{t}_l{l}"); widths.append(384)
            for t in (1, 2):
                for l in range(L):
                    names.append(f"Wup{t}_l{l}"); widths.append(128)
                    names.append(f"Ibl{t}_l{l}"); widths.append(128)
            for nm, wd in zip(names, widths):
                W[nm] = wb[:, off:off + wd]
                off += wd
            bc = cp.tile([128, 2], dt.float32, tag="bincp")
            nc.sync.dma_start(bc[:], pr["bincp"][:])
            W["binc1"] = bc[:, 0:1]
            W["binc2"] = bc[:, 1:2]
            if not bias_zero:
                for k in ("ones1", "binr1", "binr2",
                          *(f"btab{t}_l{l}" for t in (1, 2) for l in range(L)),
                          *(f"bup{t}_l{l}" for t in (1, 2) for l in range(L))):
                    p = pr[k]
                    tf2 = wp.tile(list(p.shape), dt.float32,
                                  tag="wf1" + str(list(p.shape)), bufs=2)
                    nc.sync.dma_start(tf2[:], p[:])
                    t_ = cp.tile(list(p.shape), BF, tag=k)
                    nc.vector.tensor_copy(t_[:], tf2[:])
                    W[k] = t_
        for dname in ("12", "21"):
            t_ = cp.tile([128, SJ8[dname]], dt.int16, tag=f"ei{dname}")
            nc.sync.dma_start(t_[:], pr[f"ei{dname}"][:])
            W[f"ei{dname}"] = t_
            t_ = cp.tile([128, NBLK], dt.float32, tag=f"pc{dname}")
            nc.sync.dma_start(t_[:], pr[f"pc{dname}"].rearrange("b p -> p b"))
            W[f"pc{dname}"] = t_
        t_ = cp.tile([128, NBLK * 8], dt.int16, tag="myid")
        nc.sync.dma_start(t_[:], pr["myid"][:])
        W["myid"] = t_

        def tt(eng, out, a, b, op):
            a2, b2 = broadcast_tensor_aps(a, b)
            eng.tensor_tensor(out, a2, b2, op)

        def cpy(eng, dst, src):
            if eng is nc.scalar:
                eng.copy(dst, src)
            else:
                eng.tensor_copy(dst, src)

        def gat8(out_t, table, idx_sb, base8, ntiles, elem):
            # dma_gather hardware cap: 1024 indices (8 tiles) per call
            for g0 in range(0, ntiles, 8):
                gn = min(8, ntiles - g0)
                nc.gpsimd.dma_gather(
                    out_t[:, g0:g0 + gn, :], table[:, :],
                    idx_sb[:, base8 + g0 * 8:base8 + (g0 + gn) * 8],
                    gn * 128, gn * 128, elem)

        # persistent feature tiles
        xc = {t: cp.tile([128, NP], BF, tag=f"xc{t}", name=f"xc{t}") for t in (1, 2)}
        xrow = {t: cp.tile([128, NBLK, 128], BF, tag=f"xrow{t}", name=f"xrow{t}") for t in (1, 2)}
        xnew = {t: cp.tile([128, NBLK, 128], BF, tag=f"xnew{t}", name=f"xnew{t}") for t in (1, 2)}
        qmy = {t: cp.tile([128, NBLK, 128], BF, tag=f"qmy{t}", name=f"qmy{t}") for t in (1, 2)}

        # ---------- phase 0: input projection ----------
        with tc.tile_pool(name="p0", bufs=1) as p0, \
             tc.tile_pool(name="p0b", bufs=2) as p0b, \
             tc.tile_pool(name="p0s", bufs=2) as p0s, \
             tc.tile_pool(name="p0ps", bufs=4, space="PSUM") as p0p:
            ts_ = (1, 2) if phases >= 0 else ()
            xb = {}
            for t in ts_:
                xb[t] = p0b.tile([128, NP], BF, tag="xb", name=f"xb{t}")
                for hf in range(4):
                    xf = p0.tile([128, NP // 4], dt.float32, tag="xf", bufs=2)
                    nc.sync.dma_start(xf[:], pr[f"xT{t}"][:, bass.ts(hf, NP // 4)])
                    if t == 1:
                        nc.vector.tensor_copy(xb[t][:, bass.ts(hf, NP // 4)], xf[:])
                    else:
                        nc.scalar.copy(xb[t][:, bass.ts(hf, NP // 4)], xf[:])
            for t in ts_:
                # transposed projection -> xc (x1T)
                for j in range(NP // 512):
                    ps = p0p.tile([128, 512], dt.float32, tag="psP")
                    nc.tensor.matmul(ps[:], W[f"Win{t}"], xb[t][:, bass.ts(j, 512)],
                                     start=True, stop=True)
                    nc.scalar.activation(xc[t][:, bass.ts(j, 512)], ps[:],
                                         AF.Relu, bias=W[f"binc{t}"])
            for t in ts_:
                # row projection -> xt0 table (for my-row gather)
                rstage = p0s.tile([128, NT // 2, 128], BF, tag="rst")
                for hh in range(2):
                    for i4 in range(NT // 8):
                        ps = p0p.tile([128, 4, 128], dt.float32, tag="psR")
                        for k in range(4):
                            g = hh * (NT // 2) + i4 * 4 + k
                            nc.tensor.matmul(ps[:, k, :], xb[t][:, bass.ts(g, 128)],
                                             W[f"Win{t}"], start=True,
                                             stop=bias_zero)
                            if not bias_zero:
                                nc.tensor.matmul(ps[:, k, :], W["ones1"][:1, :],
                                                 W[f"binr{t}"][:1, :],
                                                 start=False, stop=True)
                        nc.scalar.activation(
                            rstage[:, i4 * 4:(i4 + 1) * 4, :], ps[:], AF.Relu)
                    nc.sync.dma_start(
                        xt0[t].rearrange("(a p) c -> p a c", p=128)[:, hh * 40:(hh + 1) * 40, :],
                        rstage[:])
                # zero pad rows of kv
                z = p0s.tile([128, 256], BF, tag="z")
                nc.vector.memset(z[:], 0.0)
                nc.sync.dma_start(kv[t][NP:NP + 128, :], z[:])
            # gather my x0 rows
            for t in ts_:
                gat8(xrow[t], xt0[t], W["myid"], 0, NBLK, 128)

        def emit_tables(t, l):
            with tc.tile_pool(name="tab", bufs=2) as tp, \
                 tc.tile_pool(name="tabps", bufs=4, space="PSUM") as tps:
                NH = NT // 2
                for hh in range(2):
                    kvq = tp.tile([128, NH, 384], BF, tag="kvq")
                    for i in range(NH):
                        g = hh * NH + i
                        ps = tps.tile([128, 384], dt.float32, tag="psT")
                        nc.tensor.matmul(ps[:], xc[t][:, bass.ts(g, 128)],
                                         W[f"Wtab{t}_l{l}"], start=True,
                                         stop=bias_zero)
                        if not bias_zero:
                            nc.tensor.matmul(ps[:], W["ones1"][:1, :],
                                             W[f"btab{t}_l{l}"][:1, :],
                                             start=False, stop=True)
                        cpy(nc.scalar if i % 2 == 0 else nc.vector,
                            kvq[:, i, :], ps[:])
                    nc.sync.dma_start(
                        kv[t].rearrange("(a p) c -> p a c", p=128)[:, hh * NH:(hh + 1) * NH, :],
                        kvq[:, :, 0:256])
                    nc.sync.dma_start(
                        qt[t].rearrange("(a p) c -> p a c", p=128)[:, hh * NH:(hh + 1) * NH, :],
                        kvq[:, :, 256:384])
                gat8(qmy[t], qt[t], W["myid"], 0, NBLK, 128)

        for l in range(L if phases >= 1 else 0):
            # ---------- tables ----------
            # (type-2 tables of layer l>0 were emitted during layer l-1's
            # post phase, hiding the second collective)
            if l == 0:
                emit_tables(2, 0)
            emit_tables(1, l)

            # ---------- edge phase ----------
            CH = 32
            for dname, st, dtt in ((("12", 1, 2), ("21", 2, 1)) if phases >= 2 else ()):
                chunks = []
                off8 = 0
                for b in range(NBLK):
                    J = JL[dname][b]
                    nch = (J + CH - 1) // CH
                    for ci in range(nch):
                        j0 = ci * CH
                        jn = min(CH, J - j0)
                        chunks.append((b, off8 + j0 * 8, jn,
                                       ci == 0, ci == nch - 1))
                    off8 += J * 8
                with tc.tile_pool(name="eg", bufs=4) as eg, \
                     tc.tile_pool(name="epw", bufs=3) as epw, \
                     tc.tile_pool(name="ew", bufs=2) as ew, \
                     tc.tile_pool(name="eps", bufs=2, space="PSUM") as eps, \
                     tc.tile_pool(name="epsg", bufs=2, space="PSUM") as epsg, \
                     tc.tile_pool(name="epsu", bufs=2, space="PSUM") as epsu:
                    psG = sacc = None
                    for (b, coff8, jn, first, last) in chunks:
                        kvg = eg.tile([128, CH, 256], BF, tag="kvg")
                        gat8(kvg, kv[st], W[f"ei{dname}"], coff8, jn, 256)
                        prod = epw.tile([128, CH, 128], BF, tag="pw", name="prod")
                        tt(nc.vector, prod[:, 0:jn, :], kvg[:, 0:jn, 0:128],
                           qmy[dtt][:, b:b + 1, :], ALU.mult)
                        # alpha[p, j, h] = sum_d prod[p, j, d*8+h]: PE
                        # identity-matmul accumulation over the 16 d-slabs
                        psA = eps.tile([128, CH * 8], dt.float32, tag="psA")
                        for dd in range(D):
                            nc.tensor.matmul(psA[:, 0:jn * 8], W["ident"],
                                             prod[:, 0:jn, bass.ts(dd, 8)],
                                             start=(dd == 0), stop=(dd == D - 1))
                        eB = ew.tile([128, CH, 8], BF, tag="eB")
                        nc.scalar.activation(
                            eB[:, 0:jn, :],
                            psA[:, 0:jn * 8].rearrange("p (j h) -> p j h", h=8),
                            AF.Exp)
                        # wv[p, j, d*8+h] = v * e  (2x: d-major v, h innermost)
                        wv = epw.tile([128, CH, 128], BF, tag="pw", name="wv")
                        tt(nc.vector,
                           wv[:, 0:jn, :].rearrange("p j (d h) -> p j d h", h=8),
                           kvg[:, 0:jn, 128:256].rearrange("p j (d h) -> p j d h", h=8),
                           eB[:, 0:jn, :].rearrange("p j (d2 h) -> p j d2 h", d2=1),
                           ALU.mult)
                        # s tree over j (in-place on eB), bf16 accum
                        with nc.allow_low_precision(reason="softmax denom bf16 tree"):
                            Jc = jn
                            while Jc > 1:
                                h1 = (Jc + 1) // 2
                                tt(nc.vector, eB[:, 0:Jc - h1, :], eB[:, 0:Jc - h1, :],
                                   eB[:, h1:Jc, :], ALU.add)
                                Jc = h1
                        if first:
                            sacc = ew.tile([128, 8], dt.float32, tag="sacc")
                            nc.vector.tensor_copy(sacc[:], eB[:, 0, :])
                        else:
                            tt(nc.vector, sacc[:], sacc[:], eB[:, 0, :], ALU.add)
                        # agg[p, dh] += sum_j wv: 4-tile-packed identity matmuls
                        if first:
                            psG = epsg.tile([128, 4, 128], dt.float32, tag="psG")
                        nst = (jn + 3) // 4
                        for g in range(nst):
                            gw = min(4, jn - g * 4)
                            nc.tensor.matmul(psG[:, 0:gw, :], W["ident"],
                                             wv[:, g * 4:g * 4 + gw, :],
                                             start=(first and g == 0),
                                             stop=(last and g == nst - 1))
                        if not last:
                            continue
                        sden = ew.tile([128, 8], dt.float32, tag="sden")
                        tt(nc.vector, sden[:, :], sacc[:],
                           W[f"pc{dname}"][:, b:b + 1], ALU.subtract)
                        rs = ew.tile([128, 8], dt.float32, tag="rs")
                        nc.vector.reciprocal(rs[:], sden[:])
                        agf = ew.tile([128, 128], dt.float32, tag="agf")
                        nc.vector.tensor_reduce(
                            agf[:], psG[:].rearrange("p r c -> p c r"),
                            mybir.AxisListType.X, ALU.add)
                        # normalize + gelu
                        gn = ew.tile([128, 128], BF, tag="gn")
                        tt(nc.vector, gn[:].rearrange("p (d h) -> p d h", h=8),
                           agf[:].rearrange("p (d h) -> p d h", h=8),
                           rs[:].rearrange("p (d2 h) -> p d2 h", d2=1), ALU.mult)
                        gb = ew.tile([128, 128], BF, tag="gb")
                        nc.scalar.activation(gb[:], gn[:], AF.Gelu)
                        # update: x_new = gelu^T @ Wup + bup + Ibl @ x_old
                        trp = eps.tile([128, 128], BF, tag="trp")
                        nc.tensor.transpose(trp[:], gb[:], W["ident"])
                        gT = ew.tile([128, 128], BF, tag="gT")
                        nc.scalar.copy(gT[:], trp[:])
                        psU = epsu.tile([128, 128], dt.float32, tag="psU")
                        nc.tensor.matmul(psU[:], gT[:], W[f"Wup{dtt}_l{l}"],
                                         start=True, stop=False)
                        if not bias_zero:
                            nc.tensor.matmul(psU[:], W["ones1"][:1, :],
                                             W[f"bup{dtt}_l{l}"][:1, :],
                                             start=False, stop=False)
                        nc.tensor.matmul(psU[:], W[f"Ibl{dtt}_l{l}"],
                                         xrow[dtt][:, b, :], start=False, stop=True)
                        nc.vector.tensor_copy(xnew[dtt][:, b, :], psU[:])
                    nc.sync.dma_start(
                        agin[dtt].rearrange("(b p) c -> p b c", p=128),
                        xnew[dtt][:])
                # per-type allgather right after its edge dir: the second
                # dir's edge compute hides the first collective
                if phases >= 3:
                    nc.gpsimd.collective_compute(
                        "AllGather", mybir.AluOpType.bypass,
                        ins=[agin[dtt][:]], outs=[agout[dtt][:]],
                        replica_groups=[list(range(NCORE))])
            # post per type after both dirs (type 2 first: its collective
            # finished during dir 21's edge compute)
            post_types = (2, 1) if phases >= 3 else ()
            for t in post_types:
                with tc.tile_pool(name="post", bufs=1) as pp, \
                     tc.tile_pool(name="postps", bufs=4, space="PSUM") as ppp:
                    stage = pp.tile([128, NT, 128], BF, tag="stage")
                    sv = stage[:].rearrange("p (b r) c -> p b r c", r=NCORE)
                    for r in range(NCORE):
                        nc.sync.dma_start(
                            sv[:, :, r, :],
                            agout[t][r].rearrange("(b p) c -> p b c", p=128))
                    nc.sync.dma_start(
                        emed[t].rearrange("(g p) c -> p g c", p=128)[:, :, l * 128:(l + 1) * 128],
                        stage[:])
                    if l + 1 < L:
                        for q4 in range(NT // 4):
                            ptr = ppp.tile([128, 512], BF, tag="ptr")
                            for k in range(4):
                                nc.tensor.transpose(ptr[:, bass.ts(k, 128)],
                                                    stage[:, q4 * 4 + k, :], W["ident"])
                            cpy(nc.scalar if q4 % 2 == 0 else nc.vector,
                                xc[t][:, bass.ts(q4, 512)], ptr[:])
                if t == 2 and l + 1 < L:
                    # type-2 tables of the next layer: overlaps the type-1
                    # collective still in flight
                    emit_tables(2, l + 1)
                if t == 2 and l + 1 == L:
                    # final-phase index loads ahead of post-1's
                    # collective-gated DMAs, so ed-gathers can prefetch
                    # during the last collective
                    fip = stack.enter_context(tc.tile_pool(name="fidx", bufs=1))
                    for k in ("fmi", "fdi"):
                        t_ = fip.tile([128, FT * 8], dt.int16, tag=k, name=k)
                        nc.sync.dma_start(t_[:], pr[k][:])
                        W[k] = t_
            # my rows for next layer's skip = locally computed x_new
            xrow, xnew = xnew, xrow

        # ---------- final gather-dot ----------
        with tc.tile_pool(name="fin", bufs=6) as fp, \
             tc.tile_pool(name="finps", bufs=4, space="PSUM") as fps, \
             tc.tile_pool(name="ybuf", bufs=1) as yp:
            ysb = yp.tile([128, FT], dt.float32, tag="ysb")
            if phases < 4:
                nc.vector.memset(ysb[:], 0.0)
            for ch in range(NFCH if phases >= 4 else 0):
                em = fp.tile([128, FCH, 256], BF, tag="em")
                ed = fp.tile([128, FCH, 256], BF, tag="ed")
                gat8(ed, emed[2], W["fdi"], ch * FCH * 8, FCH, 256)
                gat8(em, emed[1], W["fmi"], ch * FCH * 8, FCH, 256)
                pb = fp.tile([128, FCH, 256], BF, tag="pb")
                tt(nc.vector, pb[:], em[:], ed[:], ALU.mult)
                # slab-sum on PE: psY[p, t, i] = sum_g pb[p, t, g*16+i]
                psY = fps.tile([128, FCH, 16], dt.float32, tag="psY")
                for g in range(16):
                    nc.tensor.matmul(psY[:], W["ident"],
                                     pb[:, :, bass.ts(g, 16)],
                                     start=(g == 0), stop=(g == 15))
                nc.vector.tensor_reduce(
                    ysb[:, ch * FCH:(ch + 1) * FCH], psY[:],
                    mybir.AxisListType.X, ALU.add)
            nc.sync.dma_start(y_out[:, :], ysb[:])
    nc.compile()
    return nc


_CACHE = {}


def kernel(**inputs):
    from concourse.bass_utils import run_bass_kernel_spmd
    P, e12, e21, percore, J12, J21 = _host_prep(inputs)
    bz = all(not np.any(np.asarray(inputs[k]))
             for k in inputs if k.startswith("b"))
    key = (J12, J21, bz)
    if key not in _CACHE:
        _CACHE[key] = _build(J12, J21, bias_zero=bz)
    nc = _CACHE[key]
    in_maps = []
    for c in range(NCORE):
        m = dict(P)
        m["ei12"] = e12[c]["idx"]; m["pc12"] = e12[c]["padc"]
        m["ei21"] = e21[c]["idx"]; m["pc21"] = e21[c]["padc"]
        m["myid"] = percore[c]["myid"]
        m["fmi"] = percore[c]["fmi"]; m["fdi"] = percore[c]["fdi"]
        in_maps.append(m)
    res = run_bass_kernel_spmd(nc, in_maps, list(range(NCORE)))
    ys = []
    for c in range(NCORE):
        yc = res.results[c]["y"]          # [128, FT]
        ys.append(yc.T.ravel()[:EFC])
    return np.concatenate(ys).astype(np.float32).reshape(EF, 1)
